# revision 11
# baseline (speedup 1.0000x reference)
"""Trainium2 Bass kernel for conditional-adjustment conv (CAConv), fp16.

Per sample b: h = relu(c[b] @ mlp_w1 + mlp_b1); adj = h @ mlp_w2 + mlp_b2;
w[b] = conv_w + adj.reshape(Co,Ci,3,3); out[b] = conv2d(x[b], w[b], pad=1) + conv_b.

Sharding: data-parallel over batch, 4 samples per core on 8 cores (SPMD).

All heavy matmuls in fp16 (full PE rate); psum accumulation stays fp32, so
rel err ~5e-4 << the 2e-2 budget. The host pre-casts padded x and the packed
w2 to fp16 and the kernel returns fp16 output (halves HBM traffic both
ways); the host casts back to fp32.

Per-core device kernel:
  Stage A (weight gen): ph = w1' @ c'T (fp32, tiny) -> relu+bias -> hT fp16,
  broadcast to [17, 32] (each sample column replicated 8x) and replicated at
  partition offsets 0/32/64/96. w2 is host-permuted to (ci, t, co) column
  order, packed in 4 k-groups [128, 9216] (group g = ci 16g..16g+16), with
  mlp_b2 + conv_w folded into ones-row 16. For each 512-col chunk, 4
  matmuls with tile_position=(32g, 32g) write M=32 rows each so one psum
  tile [128, 512] is fully covered -> full-width DVE/ACT copies (fp32->fp16)
  into the partition-grouped adj4 [128, 9216] (row 32g+8b+r holds sample
  b's weights for ci group g, replicated 8x). One scatter DMA per sample
  (partition-stride-32 source AP) lays the weights onto the diagonal
  blocks of the per-pair block-diag tile wblk[ci + 64*half,
  t*128 + 64*half + co]; off-diag zeros come from a GPSIMD memset.
  Stage B (conv): host-padded fp16 x (130x130) for a sample pair lives as
  [ci(2 samples), h, w] across 128 partitions. Each output chunk
  po[128, 512] (2 samples x 64 co partitions; 4 h-rows x 128 w free)
  accumulates 9 shift-tap K=128 fp16 matmuls; 6 psum bufs so the next
  group's matmuls never wait on the bias-copy drain. Bias is added during
  the PSUM->SBUF copy (alternating DVE / ACT engines) into [128, 2048]
  fp16 staging tiles; one output DMA per 16 h-rows.

  DMA queues: SP HWDGE = consts + w2 + hT bcast/repl + bulk x loads; ACT
  HWDGE = output stores (keeps them off the load queue); GPSIMD SWDGE =
  wblk memset + weight scatters.
"""

import sys

if "/opt/trn_rl_repo" not in sys.path:
    sys.path.insert(0, "/opt/trn_rl_repo")

import numpy as np

B = 32
NCORES = 8
BPC = B // NCORES          # samples per core = 4
PAIRS = BPC // 2           # sample pairs per core = 2
CIN = COUT = 64
H = W = 128
HP = WP = 130              # padded dims
KH = KW = 3
NT = KH * KW               # taps = 9
CL = 8                     # c length
CL1 = CL + 1               # + ones row
MH = 16                    # mlp hidden
K2 = MH + 1                # mlp hidden + ones row
WTOT = NT * CIN * COUT     # 36864 weights per sample
GCOL = WTOT // 4           # 9216 cols per packed w2 group
XCH = 5                    # x chunks per pair
XCHE = (HP * WP) // XCH    # 3380 elems per chunk (26 padded rows)

_CACHE = {}


def _build():
    import concourse.bass as bass
    import concourse.mybir as mybir
    import concourse.tile as tile
    from concourse import bacc

    f32 = mybir.dt.float32
    f16 = mybir.dt.float16
    AF = mybir.ActivationFunctionType

    nc = bacc.Bacc("TRN2", target_bir_lowering=False, debug=False)

    xs_d = nc.dram_tensor("xsp", [BPC, CIN, HP * WP], f16, kind="ExternalInput")
    w2_d = nc.dram_tensor("w2p", [128, GCOL], f16, kind="ExternalInput")
    cst_d = nc.dram_tensor("cst", [128, 23], f32, kind="ExternalInput")
    out_d = nc.dram_tensor("out", [BPC, COUT, H, W], f16, kind="ExternalOutput")

    with tile.TileContext(nc) as tc:
        with (
            tc.tile_pool(name="consts", bufs=1) as consts,
            tc.tile_pool(name="adjpool", bufs=1) as adjpool,
            tc.tile_pool(name="xpool", bufs=2) as xpool,
            tc.tile_pool(name="opool", bufs=6) as opool,
            tc.tile_pool(name="pspool", bufs=1, space=bass.MemorySpace.PSUM) as ps,
        ):
            # ---- constants: one packed [128, 23] f32 tile (SP queue) ----
            # cols 0-3 cT' (rows 0-8, ones row 8), 4-20 w1' (rows 0-8),
            # 21 b1 (rows 0-16), 22 conv_b tiled x2 (rows 0-127)
            cst = consts.tile([128, 23], f32)
            nc.sync.dma_start(out=cst[:], in_=cst_d.ap())
            ct_sb = cst[0:CL1, 0:BPC]
            w1_sb = cst[0:CL1, BPC : BPC + K2]
            b1_sb = cst[0:K2, 21:22]
            cb_sb = cst[:, 22:23]

            # ---- packed w2 [128, 9216] f16: 4 k-groups at partition
            # offsets 0/32/64/96; group g holds (ci, t, co)-ordered cols
            # for ci in [16g, 16(g+1)) ----
            w2s = consts.tile([128, GCOL], f16, name="w2s")
            for cc in range(4):
                nc.sync.dma_start(
                    out=w2s[:, cc * (GCOL // 4) : (cc + 1) * (GCOL // 4)],
                    in_=w2_d.ap()[:, cc * (GCOL // 4) : (cc + 1) * (GCOL // 4)],
                )

            # ---- stage A head: tiny fp32 MLP matmul + relu ----
            ph = ps.tile([K2, BPC], f32, tag="ps", bufs=8)
            nc.tensor.matmul(ph[:], w1_sb, ct_sb, start=True, stop=True)
            ht = consts.tile([K2, BPC], f16, name="ht")
            nc.scalar.activation(out=ht[:], in_=ph[:], func=AF.Relu, bias=b1_sb)
            # hT [17, 32]: the 4 sample columns tiled 8x (col m = sample
            # m%4) -> M=32 matmuls fully cover psum partitions; then
            # replicate to partition offsets 32/64/96
            ht8 = consts.tile([128, 32], f16, name="ht8")
            src8 = bass.AP(
                tensor=ht[:].tensor, offset=ht[:].offset,
                ap=[list(ht[:].ap[0]), [0, 8], [1, BPC]],
            )
            nc.sync.dma_start(out=ht8[0:K2, :], in_=src8)
            for g in range(1, 4):
                nc.sync.dma_start(
                    out=ht8[32 * g : 32 * g + K2, :], in_=ht8[0:K2, :]
                )

            # ---- bulk x loads: 5 chunks per pair (SP queue) ----
            xps = []
            for p in range(PAIRS):
                xp = xpool.tile([128, HP * WP], f16, name=f"xp{p}", tag="xp")
                xps.append(xp)
            for p in range(PAIRS):
                for k in range(XCH):
                    nc.sync.dma_start(
                        out=xps[p][:, k * XCHE : (k + 1) * XCHE],
                        in_=xs_d.ap()[2 * p : 2 * p + 2].rearrange(
                            "b c (k e) -> b c k e", e=XCHE
                        )[:, :, k, :],
                    )

            # per-pair block-diag weights; off-diag zeros via gpsimd memset
            wblk = []
            for p in range(PAIRS):
                wb = consts.tile([128, NT * 128], f16, name=f"wblk{p}", tag=f"wblk{p}")
                nc.gpsimd.memset(wb[:], 0.0)
                wblk.append(wb)

            # ---- stage A body: adj4[32g + 4r + b, c] = sample b's weight
            # for flat col 9216g + c ((ci,t,co) order), r = 0..7 replicas ----
            adj4 = adjpool.tile([128, GCOL], f16, name="adj4")
            for m in range(GCOL // 512):
                j = m * 512
                pa = ps.tile([128, 512], f32, tag="ps", bufs=8)
                for g in range(4):
                    nc.tensor.matmul(
                        pa[32 * g : 32 * g + 32, :],
                        ht8[32 * g : 32 * g + K2, :],
                        w2s[32 * g : 32 * g + K2, j : j + 512],
                        start=True,
                        stop=True,
                        tile_position=(32 * g, 32 * g),
                    )
                if m % 2 == 0:
                    nc.vector.tensor_copy(adj4[:, j : j + 512], pa[:])
                else:
                    nc.scalar.copy(adj4[:, j : j + 512], pa[:])

            # one scatter per sample: partitions [b, 32+b, 64+b, 96+b]
            # (replica 0 of each group) onto wblk's diagonal blocks.
            # Pair-1's scatters are emitted after the first conv group so
            # pair-0's scatter descriptors get the DMA engines to
            # themselves (conv start gates on them).
            def scatter(b):
                p, half = divmod(b, 2)
                q = half * 64
                dst = wblk[p][q : q + 64, :].rearrange(
                    "p (t co) -> p t co", co=128
                )[:, :, q : q + 64]
                nc.gpsimd.dma_start(out=dst, in_=adj4[b : 128 : 32, :])

            scatter(0)
            scatter(1)

            # ---- stage B: per-pair conv, 8 groups of 4 psum chunks ----
            for p in range(PAIRS):
                xp3 = xps[p].rearrange("p (h w) -> p h w", w=WP)
                for g in range(8):
                    if p == 0 and g == 1:
                        scatter(2)
                        scatter(3)
                    pos = [
                        ps.tile([128, 512], f32, tag="ps", bufs=8, name=f"po{p}_{g}_{j}")
                        for j in range(4)
                    ]
                    for t in range(NT):
                        kh, kw = divmod(t, 3)
                        for j in range(4):
                            h0 = (g * 4 + j) * 4
                            nc.tensor.matmul(
                                pos[j][:],
                                wblk[p][:, t * 128 : (t + 1) * 128],
                                xp3[:, h0 + kh : h0 + kh + 4, kw : kw + W],
                                start=(t == 0),
                                stop=(t == NT - 1),
                            )
                    os = opool.tile([128, 2048], f16, name=f"os{p}_{g}", tag="os")
                    for j in range(4):
                        if j % 2 == 0:
                            nc.vector.tensor_scalar_add(
                                os[:, j * 512 : (j + 1) * 512], pos[j][:], cb_sb
                            )
                        else:
                            nc.scalar.add(
                                os[:, j * 512 : (j + 1) * 512], pos[j][:], cb_sb
                            )
                    nc.scalar.dma_start(
                        out=out_d.ap()[2 * p : 2 * p + 2, :, 16 * g : 16 * g + 16, :],
                        in_=os[:],
                    )

    nc.compile()
    return nc


def _get_nc():
    if "nc" not in _CACHE:
        _CACHE["nc"] = _build()
    return _CACHE["nc"]


def _prep(x, c, conv_w, conv_b, mlp_w1, mlp_b1, mlp_w2, mlp_b2):
    x = np.asarray(x, dtype=np.float32)
    c = np.asarray(c, dtype=np.float32)
    conv_w = np.asarray(conv_w, dtype=np.float32)
    conv_b = np.asarray(conv_b, dtype=np.float32)
    mlp_w1 = np.asarray(mlp_w1, dtype=np.float32)
    mlp_b1 = np.asarray(mlp_b1, dtype=np.float32)
    mlp_w2 = np.asarray(mlp_w2, dtype=np.float32)
    mlp_b2 = np.asarray(mlp_b2, dtype=np.float32)

    # padded fp16 x, flattened spatial
    xsp = np.zeros((B, CIN, HP, WP), dtype=np.float16)
    xsp[:, :, 1 : HP - 1, 1 : WP - 1] = x.astype(np.float16)
    xsp = xsp.reshape(B, CIN, HP * WP)

    # w2p[k, (ci, t, co)] = mlp_w2[k, co*576 + ci*9 + t]
    # row 16 = (mlp_b2 + conv_w), same permutation -> adj == full weight
    w2p = mlp_w2.reshape(MH, COUT, CIN, NT).transpose(0, 2, 3, 1).reshape(MH, WTOT)
    b2p = mlp_b2.reshape(COUT, CIN, NT).transpose(1, 2, 0)
    cwp = conv_w.reshape(COUT, CIN, NT).transpose(1, 2, 0)  # [ci, t, co]
    row16 = (b2p + cwp).reshape(1, WTOT)
    w2p = np.concatenate([w2p, row16], axis=0)  # [17, 36864]
    # 4-group pack: rows 32g..32g+17 carry cols [9216g, 9216(g+1))
    w2pk = np.zeros((128, GCOL), dtype=np.float16)
    for g in range(4):
        w2pk[32 * g : 32 * g + K2] = w2p[:, GCOL * g : GCOL * (g + 1)].astype(
            np.float16
        )

    # packed consts [128, 23] f32 (core-invariant part)
    cstb = np.zeros((128, 23), dtype=np.float32)
    cstb[:CL, BPC : BPC + MH] = mlp_w1
    cstb[CL, BPC + MH] = 1.0
    cstb[:MH, 21] = mlp_b1
    cstb[:, 22] = np.tile(conv_b, 2)

    in_maps = []
    for i in range(NCORES):
        sl = slice(i * BPC, (i + 1) * BPC)
        cst = cstb.copy()
        cst[:CL, 0:BPC] = c[sl].T
        cst[CL, 0:BPC] = 1.0
        in_maps.append(
            {
                "xsp": np.ascontiguousarray(xsp[sl]),
                "w2p": w2pk,
                "cst": cst,
            }
        )
    return in_maps


def _run(inputs, trace=False):
    from concourse.bass_utils import run_bass_kernel_spmd

    nc = _get_nc()
    in_maps = _prep(**inputs)
    res = run_bass_kernel_spmd(
        nc, in_maps, core_ids=list(range(NCORES)), trace=trace
    )
    out = np.concatenate(
        [res.results[i]["out"].astype(np.float32) for i in range(NCORES)], axis=0
    )
    return out, res


def kernel(**inputs):
    out, _ = _run(inputs, trace=False)
    return out


# revision 12
# speedup vs baseline: 1.2644x; 1.2644x over previous
"""Trainium2 Bass kernel for conditional-adjustment conv (CAConv), fp16.

Per sample b: h = relu(c[b] @ mlp_w1 + mlp_b1); adj = h @ mlp_w2 + mlp_b2;
w[b] = conv_w + adj.reshape(Co,Ci,3,3); out[b] = conv2d(x[b], w[b], pad=1) + conv_b.

Sharding: data-parallel over batch, 4 samples per core on 8 cores (SPMD).

All heavy matmuls in fp16 (full PE rate); psum accumulation stays fp32, so
rel err ~5e-4 << the 2e-2 budget. The host pre-casts padded x and the packed
w2 to fp16 and the kernel returns fp16 output (halves HBM traffic both
ways); the host casts back to fp32.

Per-core device kernel:
  Stage A (weight gen): ph = w1' @ c'T (fp32, tiny) -> relu+bias -> hT fp16,
  tiled to [17, 32] (col m = sample m%4) and replicated at partition
  offsets 0/32/64/96. w2 is host-permuted to (ci, t, co) column order,
  packed in 4 k-groups [128, 9216] (group g = ci 16g..16g+16), with
  mlp_b2 + conv_w folded into ones-row 16. For each 512-col chunk, 4
  matmuls with tile_position=(32g, 32g) write M=32 rows each so one psum
  tile [128, 512] is fully covered -> full-width DVE/ACT copies
  (fp32->fp16) into the partition-grouped adj4 [128, 9216] (row
  32g + 4r + b = sample b, ci group g, replica r).
  Weight placement is a two-hop scatter: (1) a 64-descriptor DMA per
  sample (partition-stride-32 source) into the compact staging tile
  wst[ci + 64*half, (t, co)]; (2) a same-partition strided DVE/ACT copy
  fans each half out onto the diagonal blocks of the per-pair block-diag
  tile wblk[ci + 64*half, t*128 + 64*half + co] (off-diag zeros from a
  GPSIMD memset). This avoids the 576 tiny 128B descriptors per sample a
  direct scatter would need (measured ~16us, serializing conv start).
  Stage B (conv): host-padded fp16 x (130x130) for a sample pair lives as
  [ci(2 samples), h, w] across 128 partitions. Chunk-outer/tap-inner: each
  output chunk po[128, 512] (2 samples x 64 co partitions; 4 h-rows x
  128 w free) accumulates its 9 shift-tap K=128 fp16 matmuls
  back-to-back, then its bias-copy (alternating DVE/ACT) fires
  immediately -> psum slots (8-buf rotation) free ~13us before reuse, so
  the PE never waits on the copy drain. One output DMA per 16 h-rows.

  DMA queues: SP HWDGE = w2 + hT bcast/repl + bulk x loads; ACT HWDGE =
  output stores; GPSIMD SWDGE = consts + wblk memset + staging scatters.
"""

import sys

if "/opt/trn_rl_repo" not in sys.path:
    sys.path.insert(0, "/opt/trn_rl_repo")

import numpy as np

B = 32
NCORES = 8
BPC = B // NCORES          # samples per core = 4
PAIRS = BPC // 2           # sample pairs per core = 2
CIN = COUT = 64
H = W = 128
HP = WP = 130              # padded dims
KH = KW = 3
NT = KH * KW               # taps = 9
CL = 8                     # c length
CL1 = CL + 1               # + ones row
MH = 16                    # mlp hidden
K2 = MH + 1                # mlp hidden + ones row
WTOT = NT * CIN * COUT     # 36864 weights per sample
GCOL = WTOT // 4           # 9216 cols per packed w2 group
XCH = 5                    # x chunks per pair
XCHE = (HP * WP) // XCH    # 3380 elems per chunk (26 padded rows)

_CACHE = {}


def _build():
    import concourse.bass as bass
    import concourse.mybir as mybir
    import concourse.tile as tile
    from concourse import bacc

    f32 = mybir.dt.float32
    f16 = mybir.dt.float16
    AF = mybir.ActivationFunctionType

    nc = bacc.Bacc("TRN2", target_bir_lowering=False, debug=False)

    xs_d = nc.dram_tensor("xsp", [BPC, CIN, HP * WP], f16, kind="ExternalInput")
    w2_d = nc.dram_tensor("w2p", [128, GCOL], f16, kind="ExternalInput")
    cst_d = nc.dram_tensor("cst", [128, 23], f32, kind="ExternalInput")
    out_d = nc.dram_tensor("out", [BPC, COUT, H, W], f16, kind="ExternalOutput")

    with tile.TileContext(nc) as tc:
        with (
            tc.tile_pool(name="consts", bufs=1) as consts,
            tc.tile_pool(name="adjpool", bufs=1) as adjpool,
            tc.tile_pool(name="xpool", bufs=2) as xpool,
            tc.tile_pool(name="opool", bufs=6) as opool,
            tc.tile_pool(name="pspool", bufs=1, space=bass.MemorySpace.PSUM) as ps,
        ):
            # ---- packed w2 first on the SP queue (gates stage A) ----
            w2s = consts.tile([128, GCOL], f16, name="w2s")
            for cc in range(4):
                nc.sync.dma_start(
                    out=w2s[:, cc * (GCOL // 4) : (cc + 1) * (GCOL // 4)],
                    in_=w2_d.ap()[:, cc * (GCOL // 4) : (cc + 1) * (GCOL // 4)],
                )

            # ---- constants via GPSIMD SWDGE (off the w2/x queue) ----
            # cols 0-3 cT' (rows 0-8, ones row 8), 4-20 w1' (rows 0-8),
            # 21 b1 (rows 0-16), 22 conv_b tiled x2 (rows 0-127)
            cst = consts.tile([128, 23], f32)
            nc.gpsimd.dma_start(out=cst[:], in_=cst_d.ap())
            ct_sb = cst[0:CL1, 0:BPC]
            w1_sb = cst[0:CL1, BPC : BPC + K2]
            b1_sb = cst[0:K2, 21:22]
            cb_sb = cst[:, 22:23]

            # ---- stage A head: tiny fp32 MLP matmul + relu ----
            ph = ps.tile([K2, BPC], f32, tag="ps", bufs=8)
            nc.tensor.matmul(ph[:], w1_sb, ct_sb, start=True, stop=True)
            ht = consts.tile([K2, BPC], f16, name="ht")
            nc.scalar.activation(out=ht[:], in_=ph[:], func=AF.Relu, bias=b1_sb)
            # hT [17, 32]: the 4 sample columns tiled 8x (col m = sample
            # m%4) -> M=32 matmuls fully cover psum partitions; then
            # replicate to partition offsets 32/64/96
            ht8 = consts.tile([128, 32], f16, name="ht8")
            src8 = bass.AP(
                tensor=ht[:].tensor, offset=ht[:].offset,
                ap=[list(ht[:].ap[0]), [0, 8], [1, BPC]],
            )
            nc.sync.dma_start(out=ht8[0:K2, :], in_=src8)
            for g in range(1, 4):
                nc.sync.dma_start(
                    out=ht8[32 * g : 32 * g + K2, :], in_=ht8[0:K2, :]
                )

            # ---- bulk x loads: 5 chunks per pair (SP queue) ----
            xps = []
            for p in range(PAIRS):
                xp = xpool.tile([128, HP * WP], f16, name=f"xp{p}", tag="xp")
                xps.append(xp)
            for p in range(PAIRS):
                for k in range(XCH):
                    nc.sync.dma_start(
                        out=xps[p][:, k * XCHE : (k + 1) * XCHE],
                        in_=xs_d.ap()[2 * p : 2 * p + 2].rearrange(
                            "b c (k e) -> b c k e", e=XCHE
                        )[:, :, k, :],
                    )

            # per-pair block-diag weights; off-diag zeros via gpsimd memset
            wblk = []
            for p in range(PAIRS):
                wb = consts.tile([128, NT * 128], f16, name=f"wblk{p}", tag=f"wblk{p}")
                nc.gpsimd.memset(wb[:], 0.0)
                wblk.append(wb)

            # ---- stage A body: adj4[32g + 4r + b, c] = sample b's weight
            # for flat col 9216g + c ((ci,t,co) order), r = 0..7 replicas ----
            adj4 = adjpool.tile([128, GCOL], f16, name="adj4")
            for m in range(GCOL // 512):
                j = m * 512
                pa = ps.tile([128, 512], f32, tag="ps", bufs=8)
                for g in range(4):
                    nc.tensor.matmul(
                        pa[32 * g : 32 * g + 32, :],
                        ht8[32 * g : 32 * g + K2, :],
                        w2s[32 * g : 32 * g + K2, j : j + 512],
                        start=True,
                        stop=True,
                        tile_position=(32 * g, 32 * g),
                    )
                if m % 2 == 0:
                    nc.vector.tensor_copy(adj4[:, j : j + 512], pa[:])
                else:
                    nc.scalar.copy(adj4[:, j : j + 512], pa[:])

            # two-hop weight placement (see module docstring)
            wst = []
            for p in range(PAIRS):
                wstp = consts.tile([128, NT * 64], f16, name=f"wst{p}", tag=f"wst{p}")
                wst.append(wstp)
            for b in range(BPC):
                p, half = divmod(b, 2)
                q = half * 64
                nc.gpsimd.dma_start(
                    out=wst[p][q : q + 64, :], in_=adj4[b : 128 : 32, :]
                )
            for b in range(BPC):
                p, half = divmod(b, 2)
                q = half * 64
                dst = wblk[p][q : q + 64, :].rearrange(
                    "p (t co) -> p t co", co=128
                )[:, :, q : q + 64]
                src = wst[p][q : q + 64, :].rearrange("p (t co) -> p t co", co=64)
                if half == 0:
                    nc.vector.tensor_copy(dst, src)
                else:
                    nc.scalar.copy(dst, src)

            # ---- stage B: per-pair conv, chunk-outer / tap-inner ----
            for p in range(PAIRS):
                xp3 = xps[p].rearrange("p (h w) -> p h w", w=WP)
                for g in range(8):
                    os = opool.tile([128, 2048], f16, name=f"os{p}_{g}", tag="os")
                    for j in range(4):
                        h0 = (g * 4 + j) * 4
                        po = ps.tile(
                            [128, 512], f32, tag="ps", bufs=8, name=f"po{p}_{g}_{j}"
                        )
                        for t in range(NT):
                            kh, kw = divmod(t, 3)
                            nc.tensor.matmul(
                                po[:],
                                wblk[p][:, t * 128 : (t + 1) * 128],
                                xp3[:, h0 + kh : h0 + kh + 4, kw : kw + W],
                                start=(t == 0),
                                stop=(t == NT - 1),
                            )
                        if j % 2 == 0:
                            nc.vector.tensor_scalar_add(
                                os[:, j * 512 : (j + 1) * 512], po[:], cb_sb
                            )
                        else:
                            nc.scalar.add(
                                os[:, j * 512 : (j + 1) * 512], po[:], cb_sb
                            )
                    nc.scalar.dma_start(
                        out=out_d.ap()[2 * p : 2 * p + 2, :, 16 * g : 16 * g + 16, :],
                        in_=os[:],
                    )

    nc.compile()
    return nc


def _get_nc():
    if "nc" not in _CACHE:
        _CACHE["nc"] = _build()
    return _CACHE["nc"]


def _prep(x, c, conv_w, conv_b, mlp_w1, mlp_b1, mlp_w2, mlp_b2):
    x = np.asarray(x, dtype=np.float32)
    c = np.asarray(c, dtype=np.float32)
    conv_w = np.asarray(conv_w, dtype=np.float32)
    conv_b = np.asarray(conv_b, dtype=np.float32)
    mlp_w1 = np.asarray(mlp_w1, dtype=np.float32)
    mlp_b1 = np.asarray(mlp_b1, dtype=np.float32)
    mlp_w2 = np.asarray(mlp_w2, dtype=np.float32)
    mlp_b2 = np.asarray(mlp_b2, dtype=np.float32)

    # padded fp16 x, flattened spatial
    xsp = np.zeros((B, CIN, HP, WP), dtype=np.float16)
    xsp[:, :, 1 : HP - 1, 1 : WP - 1] = x.astype(np.float16)
    xsp = xsp.reshape(B, CIN, HP * WP)

    # w2p[k, (ci, t, co)] = mlp_w2[k, co*576 + ci*9 + t]
    # row 16 = (mlp_b2 + conv_w), same permutation -> adj == full weight
    w2p = mlp_w2.reshape(MH, COUT, CIN, NT).transpose(0, 2, 3, 1).reshape(MH, WTOT)
    b2p = mlp_b2.reshape(COUT, CIN, NT).transpose(1, 2, 0)
    cwp = conv_w.reshape(COUT, CIN, NT).transpose(1, 2, 0)  # [ci, t, co]
    row16 = (b2p + cwp).reshape(1, WTOT)
    w2p = np.concatenate([w2p, row16], axis=0)  # [17, 36864]
    # 4-group pack: rows 32g..32g+17 carry cols [9216g, 9216(g+1))
    w2pk = np.zeros((128, GCOL), dtype=np.float16)
    for g in range(4):
        w2pk[32 * g : 32 * g + K2] = w2p[:, GCOL * g : GCOL * (g + 1)].astype(
            np.float16
        )

    # packed consts [128, 23] f32 (core-invariant part)
    cstb = np.zeros((128, 23), dtype=np.float32)
    cstb[:CL, BPC : BPC + MH] = mlp_w1
    cstb[CL, BPC + MH] = 1.0
    cstb[:MH, 21] = mlp_b1
    cstb[:, 22] = np.tile(conv_b, 2)

    in_maps = []
    for i in range(NCORES):
        sl = slice(i * BPC, (i + 1) * BPC)
        cst = cstb.copy()
        cst[:CL, 0:BPC] = c[sl].T
        cst[CL, 0:BPC] = 1.0
        in_maps.append(
            {
                "xsp": np.ascontiguousarray(xsp[sl]),
                "w2p": w2pk,
                "cst": cst,
            }
        )
    return in_maps


def _run(inputs, trace=False):
    from concourse.bass_utils import run_bass_kernel_spmd

    nc = _get_nc()
    in_maps = _prep(**inputs)
    res = run_bass_kernel_spmd(
        nc, in_maps, core_ids=list(range(NCORES)), trace=trace
    )
    out = np.concatenate(
        [res.results[i]["out"].astype(np.float32) for i in range(NCORES)], axis=0
    )
    return out, res


def kernel(**inputs):
    out, _ = _run(inputs, trace=False)
    return out
